# revision 1
# baseline (speedup 1.0000x reference)
"""Trainium2 Bass kernel for multi-modal causal linear attention.

Computes: Q = MLP(m1); per modality K = MLP_m(X), V = X;
out[i] = Q_i @ sum_{j: t2[j] <= t1[i]} K_j V_j^T, summed over modalities,
first 32 features returned as (1, 4096, 32).

Sharding: 8 cores. The 22528 keys are cut into 16 time-uniform pieces
(round-robin within modality, <=1536 keys each); every core gets two pieces
in a [2 x 1536]-slot layout with a FIXED segment boundary, so the SPMD
program applies weight stack A to slot columns [0:1536) and stack B to
[1536:3072) with static instruction ranges. Keys are placed at slots
tracking rank*1536/T1 (uniform slope 8/3), making the per-chunk causal
windows identical across cores. Each core computes a partial output over
ALL queries; partials are summed on-device with an AllReduce and a single
core's (replica-identical) bf16 output shard is fetched.

Wire-minimized I/O: per-core inputs are the slotted keys (zero padding
beyond 2048 empty slots total), the two K-MLP stacks, and metadata; the
queries (m1) and Q-MLP weights ride a column-sliced AllGather so they are
shipped only once. V is derived on device by PE-transposing the key tile.

Per-core algorithm (slot space, 3072 slots, 24 chunks of 128):
 - Host computes r[j] = #queries with t1 < t2[j] and places keys at slots
   ~ r*1536/T1 within their segment.
 - Device: 3-layer MLP for Q and K packed in one 128-wide stack (block-diag
   weights, stack per static column range); chunk states G_k = K_k^T V_k
   (via PE transpose of K); two per-segment prefix chains applied per query
   chunk; exact causal mask applied on the diagonal window band via
   integer-threshold compare against an iota.
"""

import functools
from contextlib import ExitStack

import numpy as np

D = 64
T1 = 4096
SEG = 1536        # slots per segment
NSEG = SEG // 128  # 12 chunks per segment
S = 2 * SEG       # 3072 key slots per core
CQ = 128          # chunk size
NK = S // CQ      # 24 chunks
DOUT = 32
N_CORES = 8
MQ = T1 // N_CORES   # per-core m1 slice width (AllGathered on device)
WQC = (3 * D) // N_CORES  # wq column-slice width riding the AllGather

# per-core (modality, n_keys, piece_index) for segments A and B
CORE_CFG = [
    [(0, 1536, 0), (0, 1536, 1)],
    [(0, 1024, 2), (1, 1536, 0)],
    [(1, 1536, 1), (1, 1536, 2)],
    [(1, 1536, 3), (2, 1536, 0)],
    [(2, 1536, 1), (2, 1536, 2)],
    [(2, 1536, 3), (2, 1536, 4)],
    [(2, 512, 5), (3, 1536, 0)],
    [(3, 1536, 1), (3, 1024, 2)],
]
PIECES = {0: [1536, 1536, 1024], 1: [1536] * 4,
          2: [1536] * 5 + [512], 3: [1536, 1536, 1024]}
# MLP weight stack per 512-column chunk of the [queries | keys+pad] layout
STACK = [0, 0, 0, 1, 1, 1, 0, 0]


def _wlo(kk, wl):
    return max(0, (kk * 1024) // 3 - wl)


def _we(kk, wh):
    return min(T1 // CQ, ((kk + 1) * 8 + 2) // 3 + wh)


# ---------------------------------------------------------------- host prep

def _split_rr(n_keys, counts):
    """Time-uniform round-robin split of ranks 0..n-1 into pieces."""
    owners = np.concatenate([np.full(c, p) for p, c in enumerate(counts)])
    pos = np.concatenate([(np.arange(c) + 0.5) / c for c in counts])
    order = np.argsort(pos, kind="stable")
    lab = np.empty(n_keys, np.int64)
    lab[np.arange(n_keys)] = owners[order]
    return lab


def _assign_scaled(r, seg):
    """Monotone slots tracking r*SEG/T1 (keys time-sorted)."""
    n = len(r)
    j = np.arange(n, dtype=np.int64)
    tgt = (r.astype(np.int64) * SEG) // T1
    s = np.maximum.accumulate(tgt - j) + j
    return np.minimum(s, seg - n + j)


def _prep_core(pieces_kr, wks, bks, wl, wh):
    """pieces_kr: [(X_keys(n,64), r(n,)), ...] for segments A,B.
    wks/bks: weight/bias stacks for the two segments."""
    Xs = np.zeros((S, D), np.float32)
    rs = np.full(S, 10**6, np.int64)
    ok = True
    for seg, (X, r) in enumerate(pieces_kr):
        slots = _assign_scaled(r, SEG) + SEG * seg
        kk = (slots % SEG) // CQ
        ok = ok and bool(
            np.all(r >= np.maximum(0, (kk * 1024) // 3 - wl))
            and np.all(r <= 128 * np.minimum(T1 // CQ,
                                             ((kk + 1) * 8 + 2) // 3 + wh)))
        Xs[slots] = X
        rs[slots] = r

    rjw = np.zeros((128, NK), np.float32)  # [j-in-chunk, k]
    for k in range(NK):
        kk = k % NSEG
        a = _wlo(kk, wl)
        b = min(T1, 128 * _we(kk, wh))
        loc = np.clip(rs[CQ * k:CQ * (k + 1)] - a, 0, b - a)
        rjw[:, k] = loc.astype(np.float32)

    # biases: cols 0:3 = [bq; bkA], cols 3:6 = [bq; bkB] (bq filled by caller)
    bcat = np.zeros((128, 6), np.float32)
    bcat[64:, 0:3] = bks[0].T
    bcat[64:, 3:6] = bks[1].T
    meta = np.concatenate([bcat, rjw], axis=1).astype(np.float32)  # (128, 30)

    wcat = np.concatenate(
        [w.transpose(1, 0, 2).reshape(D, 3 * D) for w in wks],
        axis=1).astype(np.float16)                                 # (64, 384)
    in_map = dict(
        keys=np.ascontiguousarray(Xs.T.astype(np.float16)),        # (64, S)
        wcat=np.ascontiguousarray(wcat),
        meta=meta,
    )
    return in_map, ok


# ---------------------------------------------------------------- device build

@functools.lru_cache(maxsize=4)
def _build_nc(wl, wh):
    import concourse.bass as bass
    import concourse.tile as tile
    from concourse import bacc, mybir

    f32 = mybir.dt.float32
    f16 = mybir.dt.float16
    bf16 = mybir.dt.bfloat16
    AF = mybir.ActivationFunctionType
    OP = mybir.AluOpType

    nc = bacc.Bacc("TRN2", target_bir_lowering=False, debug=False,
                   enable_asserts=False, num_devices=N_CORES)

    m1s_d = nc.dram_tensor("m1s", [D, MQ + WQC], f16, kind="ExternalInput").ap()
    keys_d = nc.dram_tensor("keys", [D, S], f16, kind="ExternalInput").ap()
    wcat_d = nc.dram_tensor("wcat", [D, 6 * D], f16, kind="ExternalInput").ap()
    meta_d = nc.dram_tensor("meta", [128, 6 + NK], f32, kind="ExternalInput").ap()
    id_d = nc.dram_tensor("ident", [64, 64], f16, kind="ExternalInput").ap()
    out_d = nc.dram_tensor("outp", [DOUT, T1], bf16, kind="ExternalOutput").ap()

    def window(k):
        kk = k % NSEG
        return _wlo(kk, wl), min(T1, 128 * _we(kk, wh))

    nwin = max(b - a for (a, b) in (window(k) for k in range(NK)))
    nwin_ps = -(-nwin // 512) * 512   # psum tile width (bank multiples)

    def pieces(k):
        a, b_end = window(k)
        out, lo = [], a
        while lo < b_end:
            hi = min(b_end, (lo // 512 + 1) * 512)
            out.append((lo, hi))
            lo = hi
        return out

    # chunks (per segment) whose window ends at/below query chunk q
    def cnt_of(q):
        return sum(1 for kk in range(NSEG) if _we(kk, wh) <= q)

    # process segment pairs back-to-back (identical windows) so output banks
    # close as early as possible -- keeps concurrent open PSUM banks low
    korder = [NSEG * seg + kk for kk in range(NSEG) for seg in (0, 1)]
    last_mm2_pos = {}
    for pos, k in enumerate(korder):
        for (pa, pb) in pieces(k):
            last_mm2_pos[pa // 512] = pos

    with tile.TileContext(nc) as tc, ExitStack() as top:
        cpool = top.enter_context(tc.tile_pool(name="consts", bufs=1))
        hpool = top.enter_context(tc.tile_pool(name="h", bufs=1))
        spool = top.enter_context(tc.tile_pool(name="small", bufs=1))
        dpool = top.enter_context(tc.tile_pool(name="dram", bufs=1, space="DRAM"))

        # ---- AllGather the m1 column-slices (+ wq column-slices)
        m1_in = dpool.tile([D, MQ + WQC], f16, tag="m1_in", name="m1_in")
        m1_full = dpool.tile([N_CORES * D, MQ + WQC], f16, tag="m1_full",
                             name="m1_full")
        nc.gpsimd.dma_start(m1_in[:], m1s_d[:])
        nc.gpsimd.collective_compute(
            "AllGather", mybir.AluOpType.bypass,
            replica_groups=[list(range(N_CORES))],
            ins=[m1_in.opt()], outs=[m1_full.opt()])

        wqsb = cpool.tile([64, 3 * D], f16, tag="wqsb", name="wqsb")
        for s in range(N_CORES):
            nc.sync.dma_start(wqsb[:, WQC * s:WQC * (s + 1)],
                              m1_full[D * s:D * (s + 1), MQ:MQ + WQC])

        # two block-diagonal packed weight stacks assembled on device
        walls = []
        for st in range(2):
            w = cpool.tile([128, 384], f16, tag=f"wall{st}", name=f"wall{st}")
            nc.gpsimd.memset(w[:], 0.0)
            for layer in range(3):
                nc.sync.dma_start(w[0:64, 128 * layer:128 * layer + 64],
                                  wqsb[:, 64 * layer:64 * (layer + 1)])
                nc.sync.dma_start(
                    w[64:128, 128 * layer + 64:128 * (layer + 1)],
                    wcat_d[:, 192 * st + 64 * layer:192 * st + 64 * (layer + 1)])
            walls.append(w)

        metasb = cpool.tile([128, 6 + NK], f32, tag="meta", name="meta")
        nc.sync.dma_start(metasb[:], meta_d[:])
        rsb = metasb[:, 6:6 + NK]
        idsb = cpool.tile([64, 64], f16, tag="id", name="id")
        nc.sync.dma_start(idsb[:], id_d[:])
        # identity copy at base partition 64 (for transposing h0sb[64:128])
        idhi = cpool.tile([128, 64], f16, tag="idhi", name="idhi")
        nc.sync.dma_start(idhi[64:128, :], id_d[:])
        zrow = cpool.tile([1, 512], f16, tag="zrow", name="zrow")
        nc.gpsimd.memset(zrow[:], 0.0)
        iof = cpool.tile([128, nwin], f32, tag="iota", name="iota")
        nc.gpsimd.iota(iof[:], pattern=[[1, nwin]], base=0,
                       channel_multiplier=0,
                       allow_small_or_imprecise_dtypes=True)

        h0sb = hpool.tile([128, T1], f16, tag="h0", name="h0sb")
        h1 = hpool.tile([128, T1], f16, tag="h1", name="h1")
        h2 = hpool.tile([128, T1], f16, tag="h2", name="h2")
        h3 = hpool.tile([128, T1], f16, tag="h3", name="h3")
        for s in range(N_CORES):
            nc.sync.dma_start(h0sb[0:64, MQ * s:MQ * (s + 1)],
                              m1_full[D * s:D * (s + 1), 0:MQ])
        for t in range(2):
            nc.sync.dma_start(h0sb[64:128, SEG * t:SEG * (t + 1)],
                              keys_d[:, SEG * t:SEG * (t + 1)])
        nc.gpsimd.memset(h0sb[64:128, S:T1], 0.0)

        # ---- V tile derived on device: PE-transpose X chunks, keep 32 feats
        vbs = cpool.tile([128, NK * DOUT], f16, tag="vbs", name="vbs")
        with tc.tile_pool(name="psum_v", bufs=2, space="PSUM") as pv:
            for g in range(NK // 8):
                psv = pv.tile([128, 512], f16, tag="v", name="vps")
                for j in range(8):
                    k = 8 * g + j
                    nc.tensor.matmul(psv[:, 64 * j:64 * (j + 1)],
                                     h0sb[64:128, CQ * k:CQ * (k + 1)],
                                     idhi[64:128, :], is_transpose=True,
                                     start=(j == 0), stop=(j == 7))
                src = psv[:].rearrange("p (j c) -> p j c", c=64)[:, :, 0:DOUT]
                dst = vbs[:, 256 * g:256 * (g + 1)].rearrange(
                    "p (j c) -> p j c", c=DOUT)
                if g % 2 == 0:
                    nc.scalar.copy(dst, src)
                else:
                    nc.vector.tensor_copy(dst, src)

        # ---- MLP (per-512-chunk stack selection; evacs alternate ACT/DVE)
        hs = [h0sb, h1, h2, h3]
        with tc.tile_pool(name="psum_mlp", bufs=3, space="PSUM") as pmlp:
            for layer in range(3):
                src_t, dst = hs[layer], hs[layer + 1]
                for t in range(8):
                    st = STACK[t]
                    ps = pmlp.tile([128, 512], f32, tag="mlp", name="mlpps")
                    c0 = 512 * t
                    nc.tensor.matmul(ps[:], walls[st][:, 128 * layer:128 * (layer + 1)],
                                     src_t[:, c0:c0 + 512],
                                     start=True, stop=True)
                    dcol = dst[:, c0:c0 + 512]
                    bias = metasb[:, 3 * st + layer:3 * st + layer + 1]
                    if t % 2 == 0:
                        func = AF.Relu if layer < 2 else AF.Identity
                        nc.scalar.activation(dcol, ps[:], func, bias=bias)
                    elif layer < 2:
                        nc.vector.tensor_scalar(dcol, ps[:], bias, 0.0,
                                                OP.add, OP.max)
                    else:
                        nc.vector.tensor_scalar(dcol, ps[:], bias, None,
                                                OP.add)
        q_sb = h3[0:64, :]
        hkb = spool.tile([64, S], f16, tag="hkb", name="hkb")
        for t in range(2):
            nc.sync.dma_start(hkb[:, SEG * t:SEG * (t + 1)],
                              h3[64:128, SEG * t:SEG * (t + 1)])

        # ---- transpose K chunks to key-major (f16)
        km = spool.tile([128, NK * D], f16, tag="km", name="km")
        with tc.tile_pool(name="psum_t", bufs=2, space="PSUM") as pt:
            for g in range(NK // 8):
                pst = pt.tile([128, 512], f16, tag="t", name="tps")
                for j in range(8):
                    k = 8 * g + j
                    nc.tensor.matmul(pst[:, 64 * j:64 * (j + 1)],
                                     hkb[:, CQ * k:CQ * (k + 1)],
                                     idsb[:], is_transpose=True,
                                     start=(j == 0), stop=(j == 7))
                if g % 2 == 0:
                    nc.scalar.copy(km[:, 512 * g:512 * (g + 1)], pst[:])
                else:
                    nc.vector.tensor_copy(km[:, 512 * g:512 * (g + 1)], pst[:])

        # ---- chunk states G_k = K_k^T V_k, two per-segment prefix chains
        gall = spool.tile([64, NK * DOUT], f32, tag="gall", name="gall")
        with tc.tile_pool(name="psum_g", bufs=1, space="PSUM") as pg:
            psg = pg.tile([64, NK * DOUT], f32, tag="g", name="gps")
            for k in range(NK):
                nc.tensor.matmul(psg[:, DOUT * k:DOUT * (k + 1)],
                                 km[:, D * k:D * (k + 1)],
                                 vbs[:, DOUT * k:DOUT * (k + 1)],
                                 start=(k % NSEG == 0),
                                 stop=(k % NSEG == NSEG - 1))
            nc.scalar.copy(gall[:], psg[:])

        # ss{A,B}[m] = sum of first m+1 chunk states of the segment
        ssf = [spool.tile([64, NSEG * DOUT], f32, tag=f"ss{seg}",
                          name=f"ss{seg}") for seg in range(2)]
        for seg in range(2):
            g0 = NSEG * DOUT * seg
            nc.vector.tensor_copy(ssf[seg][:, 0:DOUT],
                                  gall[:, g0:g0 + DOUT])
            for m in range(1, NSEG):
                nc.vector.tensor_tensor(
                    ssf[seg][:, DOUT * m:DOUT * (m + 1)],
                    ssf[seg][:, DOUT * (m - 1):DOUT * m],
                    gall[:, g0 + DOUT * m:g0 + DOUT * (m + 1)], OP.add)
        ssb = [spool.tile([64, NSEG * DOUT], f16, tag=f"ssb{seg}",
                          name=f"ssb{seg}") for seg in range(2)]
        for seg in range(2):
            nc.vector.tensor_copy(ssb[seg][:], ssf[seg][:])

        # ---- attention
        outsb = spool.tile([DOUT, T1], f32, tag="outsb", name="outsb")
        bounce = dpool.tile([DOUT, T1], f32, tag="bounce", name="bounce")
        with tc.tile_pool(name="psum_at", bufs=2, space="PSUM") as pat, \
             tc.tile_pool(name="psum_out", bufs=3, space="PSUM") as pout, \
             tc.tile_pool(name="attn_sb", bufs=3) as apool:
            bank_tile = {}
            evac_ct = 0

            def get_bank(b):
                if b not in bank_tile:
                    t = pout.tile([DOUT, 512], f32, tag="ob", name="ob")
                    bank_tile[b] = t
                    nc.tensor.matmul(t[:], zrow[0:1, 0:DOUT], zrow[0:1, :],
                                     start=True, stop=False,
                                     skip_group_check=True)
                return bank_tile[b]

            def close_bank(b):
                nonlocal evac_ct
                # prefix-state matmuls for the 4 query chunks of this bank
                for q in range(4 * b, 4 * b + 4):
                    cnt = cnt_of(q)
                    if cnt < 1:
                        continue
                    rhs = q_sb[:, CQ * q:CQ * (q + 1)]
                    for seg in range(2):
                        lhsT = ssb[seg][:, DOUT * (cnt - 1):DOUT * cnt]
                        nc.tensor.matmul(
                            get_bank(b)[:, 128 * (q % 4):128 * (q % 4) + CQ],
                            lhsT, rhs, start=False, stop=False,
                            skip_group_check=True)
                dstc = outsb[:, 512 * b:512 * (b + 1)]
                if evac_ct % 2 == 0:
                    nc.scalar.copy(dstc, bank_tile[b][:])
                else:
                    nc.vector.tensor_copy(dstc, bank_tile[b][:])
                evac_ct += 1
                nc.gpsimd.dma_start(bounce[:, 512 * b:512 * (b + 1)], dstc)
                del bank_tile[b]

            for pos, k in enumerate(korder):
                a, b_end = window(k)
                nw = b_end - a
                atps = pat.tile([128, nwin_ps], f32, tag="at", name="atps")
                for w0 in range(0, nw, 512):
                    w1 = min(nw, w0 + 512)
                    nc.tensor.matmul(atps[:, w0:w1],
                                     hkb[:, CQ * k:CQ * (k + 1)],
                                     q_sb[:, a + w0:a + w1],
                                     start=True, stop=True)
                msk = apool.tile([128, nwin], f32, tag="msk", name="msk")
                nc.gpsimd.tensor_scalar(msk[:, 0:nw], iof[:, 0:nw],
                                        rsb[:, k:k + 1], None, OP.is_ge)
                am = apool.tile([128, nwin], f16, tag="am", name="am")
                nc.vector.tensor_tensor(am[:, 0:nw], atps[:, 0:nw],
                                        msk[:, 0:nw], OP.mult)
                for (pa, pb) in pieces(k):
                    b = pa // 512
                    nc.tensor.matmul(
                        get_bank(b)[:, pa - 512 * b:pb - 512 * b],
                        vbs[:, DOUT * k:DOUT * (k + 1)],
                        am[:, pa - a:pb - a],
                        start=False, stop=False, skip_group_check=True)
                for b in range(8):
                    if last_mm2_pos[b] == pos:
                        close_bank(b)

        # ---- on-device sum of the 8 per-core partials (f32), emit bf16
        bounce_out = dpool.tile([DOUT, T1], f32, tag="bounce_out",
                                name="bounce_out")
        nc.gpsimd.collective_compute(
            "AllReduce", mybir.AluOpType.add,
            replica_groups=[list(range(N_CORES))],
            ins=[bounce.opt()], outs=[bounce_out.opt()])
        osum = spool.tile([DOUT, T1], f32, tag="osum", name="osum")
        nc.sync.dma_start(osum[:], bounce_out[:])
        obb = spool.tile([DOUT, T1], bf16, tag="obb", name="obb")
        nc.vector.tensor_copy(obb[:], osum[:])
        nc.sync.dma_start(out_d[:], obb[:])

    nc.compile()
    return nc


# ---------------------------------------------------------------- runner

_RUNNER_CACHE = {}


def _get_runner(wl, wh):
    if (wl, wh) in _RUNNER_CACHE:
        return _RUNNER_CACHE[(wl, wh)]

    import jax
    from jax.sharding import Mesh, NamedSharding, PartitionSpec
    from jax.experimental.shard_map import shard_map
    from concourse import bass2jax, mybir

    nc = _build_nc(wl, wh)
    bass2jax.install_neuronx_cc_hook()

    partition_name = (nc.partition_id_tensor.name
                      if nc.partition_id_tensor else None)
    in_names, out_names, out_avals = [], [], []
    for alloc in nc.m.functions[0].allocations:
        if not isinstance(alloc, mybir.MemoryLocationSet):
            continue
        name = alloc.memorylocations[0].name
        if alloc.kind == "ExternalInput":
            if name != partition_name:
                in_names.append(name)
        elif alloc.kind == "ExternalOutput":
            out_names.append(name)
            out_avals.append(jax.core.ShapedArray(
                tuple(alloc.tensor_shape), mybir.dt.np(alloc.dtype)))
    n_params = len(in_names)
    in_names_all = in_names + out_names
    if partition_name is not None:
        in_names_all.append(partition_name)

    REPLICATED = {"ident"}

    def _body(*args):
        operands = list(args)
        if partition_name is not None:
            operands.append(bass2jax.partition_id_tensor())
        return tuple(bass2jax._bass_exec_p.bind(
            *operands, out_avals=tuple(out_avals),
            in_names=tuple(in_names_all), out_names=tuple(out_names),
            lowering_input_output_aliases=(), sim_require_finite=True,
            sim_require_nnan=True, nc=nc))

    devices = jax.devices()[:N_CORES]
    mesh = Mesh(np.asarray(devices), ("core",))
    P = PartitionSpec
    in_specs = tuple(P() if n in REPLICATED else P("core") for n in in_names)
    in_specs = in_specs + (P("core"),) * len(out_names)
    out_specs = (P("core"),) * len(out_names)
    sharded = jax.jit(
        shard_map(_body, mesh=mesh, in_specs=in_specs, out_specs=out_specs,
                  check_rep=False),
        keep_unused=True)

    # persistent device-resident constants: the identity tile (an
    # implementation detail, not input data) and the dummy output operands
    # (the kernel writes every output element, so their content is unused).
    ident_dev = jax.device_put(np.eye(64, dtype=np.float16),
                               NamedSharding(mesh, P()))
    dummies = [jax.device_put(
        np.zeros((N_CORES * av.shape[0], *av.shape[1:]), av.dtype),
        NamedSharding(mesh, P("core"))) for av in out_avals]

    def run(per_core_maps):
        args = []
        for name in in_names:
            if name == "ident":
                args.append(ident_dev)
            else:
                args.append(np.concatenate(
                    [m[name] for m in per_core_maps], axis=0))
        out = sharded(*args, *dummies)
        # AllReduce makes all per-core outputs identical; fetch one shard.
        return np.asarray(out[0].addressable_shards[0].data).astype(np.float32)

    _RUNNER_CACHE[(wl, wh)] = run
    return run


# ---------------------------------------------------------------- entry point

def _pick_params(inputs):
    """Choose (wl, wh) from the data; returns params + per-core input maps."""
    m1 = np.asarray(inputs["m1"], np.float32)[0, 0]         # (T1, 64)
    t1 = m1[:, -1]
    wq = np.asarray(inputs["WQ_w"], np.float32)
    bq = np.asarray(inputs["WQ_b"], np.float32)
    wk = np.asarray(inputs["WK_w"], np.float32)
    bk = np.asarray(inputs["WK_b"], np.float32)
    xs = [np.asarray(inputs[f"m{i+1}"], np.float32)[0, 0] for i in range(4)]

    # time-uniform piece split per modality
    piece_data = {}
    for mod in range(4):
        X = xs[mod]
        r = np.searchsorted(t1, X[:, -1], side="left").astype(np.int64)
        lab = _split_rr(len(r), PIECES[mod])
        for p in range(len(PIECES[mod])):
            sel = lab == p
            piece_data[(mod, p)] = (X[sel], r[sel])

    m1T = np.ascontiguousarray(m1.T.astype(np.float16))     # (64, T1)
    wq_p = wq.transpose(1, 0, 2).reshape(D, 3 * D).astype(np.float16)

    for wl, wh in [(64, 1), (128, 1), (192, 1), (256, 1)]:
        maps = []
        all_ok = True
        for c, segs in enumerate(CORE_CFG):
            pieces_kr = [piece_data[(mod, p)] for (mod, n, p) in segs]
            wks = [wk[mod] for (mod, n, p) in segs]
            bks = [bk[mod] for (mod, n, p) in segs]
            im, ok = _prep_core(pieces_kr, wks, bks, wl, wh)
            im["meta"][0:64, 0:3] = bq.T
            im["meta"][0:64, 3:6] = bq.T
            m1s = np.empty((D, MQ + WQC), np.float16)
            m1s[:, 0:MQ] = m1T[:, MQ * c:MQ * (c + 1)]
            m1s[:, MQ:] = wq_p[:, WQC * c:WQC * (c + 1)]
            im["m1s"] = m1s
            maps.append(im)
            all_ok = all_ok and ok
        if all_ok:
            return wl, wh, maps
    raise RuntimeError("no window parameterization fits the data")


def kernel(**inputs) -> np.ndarray:
    wl, wh, in_maps = _pick_params(inputs)
    run = _get_runner(wl, wh)
    total = run(in_maps)                                    # (DOUT, T1) f32
    return np.ascontiguousarray(total.T, dtype=np.float32)[None]

